# revision 15
# baseline (speedup 1.0000x reference)
"""Trainium2 Bass kernel for dual-branch (hifi windowed + lofi downsampled-KV)
attention. Data-parallel over batch: 8 batches -> 8 NeuronCores.

v5: fully software-pipelined around the ACT exp stream (~19M softmax elements
at 1 elem/lane/cycle is the per-core floor).

  Phase 0: DMA all x; 2x2 avg-pool (split DVE/gpsimd); hifi qk + V^T + lofi q
           for tile 0 (PE warm-up, only needs x); lofi k and V^T.
  Phase 1 (per 512-pixel tile nt): two lofi attention blocks (head pairs
           packed into [128,1024] PSUM via concurrent row-group matmuls, ONE
           exp per key chunk, attn@V lagged one chunk behind exp), with next
           tile's hifi/lofi projections and the previous tile's lofi output
           projection sprinkled between chunks as PE filler.
  Phase 2 (per 512-pixel block, pair-inner): hifi windowed attention
           (pair-packed logits+mask, one exp per block, attn@V and the
           normalize chain lagged one block), with the hifi output projection
           lagged one block-pair; projection evacuations on ACT (idle here).

  Softmax denominators ride as a ones-column in the V^T weights -> row 64 of
  the attn@V PSUM; packed via DMA transpose -> 128-lane reciprocal -> DMA
  back -> gpsimd partition_broadcast -> DVE normalize (fused with the hifi
  (g,i)->(h,w) scatter), writing pair-packed [128,N] tiles so the output
  projections contract over the full 128 partitions.
"""
import sys

sys.path.insert(0, "/opt/trn_rl_repo")

import numpy as np
import ml_dtypes

import concourse.bass as bass
import concourse.bacc as bacc
import concourse.mybir as mybir
import concourse.tile as tile
from concourse.bass_utils import run_bass_kernel_spmd

F32 = mybir.dt.float32
BF16 = mybir.dt.bfloat16
AF = mybir.ActivationFunctionType
MUL = mybir.AluOpType.mult

SCALE = 64 ** -0.5   # 0.125
N = 4096
M = 1024
CBIG = 320.0         # mask magnitude pre-scale (C/SCALE with C=40)

_CACHE = {}


def _build_bass():
    nc = bacc.Bacc("TRN2", target_bir_lowering=False, debug=False, num_devices=8)

    d = {}
    d["x_d"] = nc.dram_tensor("x", (512, N), BF16, kind="ExternalInput").ap()
    for nm, shp, dt in [
        ("wqk", (512, 512), BF16), ("whv", (512, 260), BF16),
        ("wlq", (512, 256), BF16), ("wlk", (512, 256), BF16),
        ("wlv", (512, 260), BF16), ("whp", (128, 512), BF16),
        ("wlp", (128, 512), BF16),
        ("smallf", (128, 12), F32), ("smallb", (128, 1160), BF16),
    ]:
        d[nm + "_d"] = nc.dram_tensor(nm, shp, dt, kind="ExternalInput").ap()
    d["y_d"] = nc.dram_tensor("y", (512, N), F32, kind="ExternalOutput").ap()

    with tile.TileContext(nc) as tc:
        _emit(nc, tc, d)
    nc.finalize()
    return nc


def _emit(nc, tc, d):
    x_d = d["x_d"]; y_d = d["y_d"]

    # ---- persistent: weights ----
    wp = tc.alloc_tile_pool(name="wp", bufs=1)
    wqk = wp.tile([128, 512 * 4], BF16, tag="wqk", name="wqk")
    whv = wp.tile([128, 260 * 4], BF16, tag="whv", name="whv")
    wlq = wp.tile([128, 256 * 4], BF16, tag="wlq", name="wlq")
    wlk = wp.tile([128, 256 * 4], BF16, tag="wlk", name="wlk")
    wlv = wp.tile([128, 260 * 4], BF16, tag="wlv", name="wlv")
    whp = wp.tile([128, 512], BF16, tag="whp", name="whp")
    wlp = wp.tile([128, 512], BF16, tag="wlp", name="wlp")
    bsf = wp.tile([128, 12], F32, tag="bsf", name="bsf")
    bsb = wp.tile([128, 1160], BF16, tag="bsb", name="bsb")
    ones1 = wp.tile([1, 128], BF16, tag="ones1", name="ones1")
    bqk = bsf[:, 0:4]; blq = bsf[:, 4:6]; blk = bsf[:, 6:8]
    bhp = bsf[:, 8:10]; blp = bsf[:, 10:12]
    mskL = bsb[:, 0:128]; mskR = bsb[:, 128:640]
    bhv = bsb[0:1, 640:900]; blv = bsb[0:1, 900:1160]
    bhv_bc = wp.tile([128, 260], BF16, tag="bhv_bc", name="bhv_bc")
    blv_bc = wp.tile([128, 260], BF16, tag="blv_bc", name="blv_bc")


    # ---- persistent pools, ordered by release time (LIFO stack) ----
    opl = tc.alloc_tile_pool(name="oplofi", bufs=1)       # lives until after lofi proj
    lq = [opl.tile([128, N], BF16, tag=f"lq{p}", name=f"lq{p}") for p in range(2)]
    lk = [opl.tile([128, M], BF16, tag=f"lk{p}", name=f"lk{p}") for p in range(2)]
    val = opl.tile([128, 260 * 8], BF16, tag="val", name="val")
    norm_l = [opl.tile([128, N], BF16, tag=f"nl{p}", name=f"nl{p}") for p in range(2)]
    oph = tc.alloc_tile_pool(name="ophifi", bufs=1)       # lives until after hifi proj
    qkh = [oph.tile([128, N], BF16, tag=f"qkh{p}", name=f"qkh{p}") for p in range(4)]
    vah = oph.tile([128, 260 * 32], BF16, tag="vah", name="vah")
    norm_h = [oph.tile([128, N], BF16, tag=f"nh{p}", name=f"nh{p}") for p in range(2)]
    opx = tc.alloc_tile_pool(name="opx", bufs=1)          # x tiles; through phase 1
    xbig = [opx.tile([128, N], BF16, tag=f"xb{kt}", name=f"xb{kt}") for kt in range(4)]
    xpb = [opx.tile([128, M], BF16, tag=f"xpb{kt}", name=f"xpb{kt}") for kt in range(4)]
    # DMA order matters: the Sync engine serializes dma_start issues at
    # ~0.7us each and the DMA engines drain ~in issue order.  Small bias/mask
    # tensors first (ACT's first evacuations need bqk), then the warm-up
    # weights, then the big x transfers, then the weights not needed until
    # later.
    nc.sync.dma_start(bsf[:], d["smallf_d"][:, :])
    nc.sync.dma_start(bsb[:], d["smallb_d"][:, :])
    for (t, nm) in [(wqk, "wqk"), (whv, "whv"), (wlq, "wlq")]:
        dr = d[nm + "_d"]
        w = dr.shape[1]
        nc.sync.dma_start(t[:].rearrange("p (k w) -> p k w", k=4),
                          dr.rearrange("(k p) w -> p k w", k=4))
    for kt in range(4):
        nc.sync.dma_start(xbig[kt][:], x_d[kt * 128:(kt + 1) * 128, :])
    xb = [[xbig[kt][:, nt * 512:(nt + 1) * 512] for kt in range(4)] for nt in range(8)]
    for (t, nm) in [(wlk, "wlk"), (wlv, "wlv")]:
        dr = d[nm + "_d"]
        w = dr.shape[1]
        nc.sync.dma_start(t[:].rearrange("p (k w) -> p k w", k=4),
                          dr.rearrange("(k p) w -> p k w", k=4))
    for (t, nm) in [(whp, "whp"), (wlp, "wlp")]:
        nc.sync.dma_start(t[:], d[nm + "_d"][:, :])
    nc.vector.memset(ones1[:], 1.0)
    nc.gpsimd.partition_broadcast(bhv_bc[:], bsb[0:1, 640:900])
    nc.gpsimd.partition_broadcast(blv_bc[:], bsb[0:1, 900:1160])

    # ============ single merged pipeline: one PSUM footprint ============
    # psA (2 banks) projection groups | sg-tag (4 banks) lofi S / hifi logits
    # | oac-tag (2 banks) lofi attn@V acc / hifi attn@V.  ACT runs the exp
    # wall; everything else is filler around it.
    with tc.tile_pool(name="psA", bufs=2, space="PSUM") as psA, \
         tc.tile_pool(name="lS", bufs=2, space="PSUM") as lS_p, \
         tc.tile_pool(name="lO", bufs=1, space="PSUM") as lO_p, \
         tc.tile_pool(name="p0t1", bufs=2) as t1_p, \
         tc.tile_pool(name="lexp", bufs=4) as le_p, \
         tc.tile_pool(name="lstg", bufs=2) as ls_p, \
         tc.tile_pool(name="lpk", bufs=2) as lpk_p, \
         tc.tile_pool(name="lrb", bufs=2) as lrb_p, \
         tc.tile_pool(name="lpyb", bufs=2) as lyb_p:

        def qk_group(nt, mt, warm=False):
            ps = psA.tile([128, 512], F32, tag="ps", name="ps")
            for kt in range(4):
                nc.tensor.matmul(ps[:], wqk[:, kt * 512 + mt * 128: kt * 512 + (mt + 1) * 128],
                                 xb[nt][kt], start=(kt == 0), stop=(kt == 3))
            if warm:
                nc.scalar.activation(qkh[mt][:, nt * 512:(nt + 1) * 512], ps[:],
                                     AF.Identity, bias=bsf[:, mt:mt + 1], scale=1.0)
            else:
                nc.vector.tensor_scalar_add(qkh[mt][:, nt * 512:(nt + 1) * 512], ps[:],
                                            bsf[:, mt:mt + 1])

        def hv_group(nt, sc, warm=False):
            st = nt * 4 + sc
            ps = psA.tile([128, 512], F32, tag="ps", name="ps")
            for kt in range(4):
                nc.tensor.matmul(ps[:, 0:260], xbig[kt][:, nt * 512 + sc * 128: nt * 512 + (sc + 1) * 128],
                                 whv[:, kt * 260:(kt + 1) * 260], start=(kt == 0),
                                 stop=(kt == 3 and not warm))
            if warm:
                nc.tensor.matmul(ps[:, 0:260], ones1[:], bsb[0:1, 640:900],
                                 start=False, stop=True, skip_group_check=True)
                nc.scalar.activation(vah[:, st * 260:(st + 1) * 260], ps[:, 0:260],
                                     AF.Identity, scale=1.0)
            else:
                nc.vector.tensor_tensor(vah[:, st * 260:(st + 1) * 260], ps[:, 0:260],
                                        bhv_bc[:], mybir.AluOpType.add)

        def lq_group(nt, mt, warm=False):
            ps = psA.tile([128, 512], F32, tag="ps", name="ps")
            for kt in range(4):
                nc.tensor.matmul(ps[:], wlq[:, kt * 256 + mt * 128: kt * 256 + (mt + 1) * 128],
                                 xb[nt][kt], start=(kt == 0), stop=(kt == 3))
            if warm:
                nc.scalar.activation(lq[mt][:, nt * 512:(nt + 1) * 512], ps[:],
                                     AF.Identity, bias=bsf[:, 4 + mt:5 + mt], scale=1.0)
            else:
                nc.vector.tensor_scalar_add(lq[mt][:, nt * 512:(nt + 1) * 512], ps[:],
                                            bsf[:, 4 + mt:5 + mt])

        def lproj_group(nt, mt):
            ps = psA.tile([128, 512], F32, tag="ps", name="ps")
            for p in range(2):
                nc.tensor.matmul(ps[:],
                                 wlp[:, p * 256 + mt * 128: p * 256 + (mt + 1) * 128],
                                 norm_l[p][:, nt * 512:(nt + 1) * 512],
                                 start=(p == 0), stop=(p == 1))
            yb = lyb_p.tile([128, 512], F32, tag="yb", name="yb")
            nc.vector.tensor_scalar_add(yb[:], ps[:], bsf[:, 10 + mt:11 + mt])
            nc.sync.dma_start(y_d[256 + mt * 128: 256 + (mt + 1) * 128,
                                  nt * 512:(nt + 1) * 512], yb[:])

        def hproj_group(nt, mt):
            ps = psA.tile([128, 512], F32, tag="ps", name="ps")
            for p in range(2):
                nc.tensor.matmul(ps[:],
                                 whp[:, p * 256 + mt * 128: p * 256 + (mt + 1) * 128],
                                 norm_h[p][:, nt * 512:(nt + 1) * 512],
                                 start=(p == 0), stop=(p == 1))
            yb = lyb_p.tile([128, 512], F32, tag="yb", name="yb")
            nc.vector.tensor_scalar_add(yb[:], ps[:], bsf[:, 8 + mt:9 + mt])
            nc.sync.dma_start(y_d[mt * 128:(mt + 1) * 128, nt * 512:(nt + 1) * 512], yb[:])

        def den_norm(stg, dst_norm, p, col0, scatter):
            # pack denominators -> 128-lane reciprocal -> broadcast -> multiply
            dpk = lpk_p.tile([128, 8], F32, tag="dpk", name="dpk")
            nc.sync.dma_start(dpk[:], stg[64:65, :].rearrange("o (p f) -> o p f", f=8))
            rpk = lpk_p.tile([128, 8], F32, tag="rpk", name="rpk")
            nc.vector.reciprocal_approx_fast(rpk[:], dpk[:])
            rrow = lpk_p.tile([1, 1024], F32, tag="rrow", name="rrow")
            nc.sync.dma_start(rrow[:].rearrange("o (p f) -> o p f", f=8), rpk[:])
            rb = lrb_p.tile([64, 1024], F32, tag="rb", name="rb")
            nc.gpsimd.partition_broadcast(rb[:], rrow[:])
            for i in range(2):
                if scatter:
                    dstv = dst_norm[64 * i:64 * i + 64, col0:col0 + 512].rearrange(
                        "p (s i1 gi2) -> p s i1 gi2", s=4, i1=2, gi2=64)
                    srcv = stg[0:64, i * 512:(i + 1) * 512].rearrange(
                        "p (s g i1 i2) -> p s g i1 i2", s=4, g=32, i1=2, i2=2)
                    rbv = rb[:, i * 512:(i + 1) * 512].rearrange(
                        "p (s g i1 i2) -> p s g i1 i2", s=4, g=32, i1=2, i2=2)
                    for i1 in range(2):
                        nc.vector.tensor_tensor(dstv[:, :, i1, :], srcv[:, :, :, i1, :],
                                                rbv[:, :, :, i1, :], MUL)
                else:
                    nc.vector.tensor_tensor(dst_norm[64 * i:64 * i + 64, col0:col0 + 512],
                                            stg[0:64, i * 512:(i + 1) * 512],
                                            rb[:, i * 512:(i + 1) * 512], MUL)

        def hifi_block(sgh, p):
            qt_, kt_ = qkh[p], qkh[2 + p]
            Lg = lS_p.tile([128, 1024], F32, tag="sg", name="Lg")
            for s4 in range(4):
                st = sgh * 4 + s4
                for i in range(2):
                    r = 64 * i
                    co = i * 512 + s4 * 128
                    nc.tensor.matmul(Lg[:, co:co + 128],
                                     kt_[r:r + 64, st * 128:(st + 1) * 128],
                                     qt_[r:r + 64, st * 128:(st + 1) * 128],
                                     start=True, stop=False)
                    nc.tensor.matmul(Lg[:, co:co + 128],
                                     bsb[r:r + 64, 0:128], bsb[r:r + 64, 128:256],
                                     start=False, stop=True)
            eL = le_p.tile([128, 1024], BF16, tag="eS", name="eL")
            nc.scalar.activation(eL[:], Lg[:], AF.Exp, scale=SCALE)
            return eL

        def hifi_tail(sgh, p, eL):
            oh = lO_p.tile([65, 1024], F32, tag="oac", name="oh")
            for s4 in range(4):
                st = sgh * 4 + s4
                for i in range(2):
                    h = 2 * p + i
                    co = i * 512 + s4 * 128
                    nc.tensor.matmul(oh[:, co:co + 128],
                                     vah[:, st * 260 + h * 65: st * 260 + (h + 1) * 65],
                                     eL[:, co:co + 128], start=True, stop=True)
            stg = ls_p.tile([65, 1024], F32, tag="stg", name="stg")
            nc.vector.tensor_copy(stg[:], oh[:])
            den_norm(stg, norm_h[p], p, sgh * 512, scatter=True)

        # ---------- Phase 0 ----------
        # 2x2 avg-pool in bf16 (the /4 folded into wlk/wlv); split DVE/gpsimd
        for nt in range(8):
            for kt in range(4):
                eng = nc.vector if (nt * 4 + kt) % 8 < 5 else nc.gpsimd
                v = xb[nt][kt][:].rearrange("p (h w2 two) -> p h w2 two", w2=32, two=2)
                t1 = t1_p.tile([128, 256], BF16, tag=f"t1{kt % 2}", name="t1")
                t1v = t1[:].rearrange("p (h w2) -> p h w2", w2=32)
                eng.tensor_add(t1v, v[:, :, :, 0], v[:, :, :, 1])
                t1p = t1[:].rearrange("p (i two w2) -> p i two w2", two=2, w2=32)
                xpv = xpb[kt][:, nt * 128:(nt + 1) * 128].rearrange("p (i w2) -> p i w2", w2=32)
                eng.tensor_add(xpv, t1p[:, :, 0, :], t1p[:, :, 1, :])
        # PE warm-up (minimal: just what tile 0's lofi blocks need; the rest
        # becomes early fillers). Evacuate via ACT: DVE is busy pooling.
        for mt in range(4):
            qk_group(0, mt, warm=True)
        for mt in range(2):
            lq_group(0, mt, warm=True)
        # lofi k
        for mt in range(2):
            for ntk in range(2):
                ps = psA.tile([128, 512], F32, tag="ps", name="ps")
                for kt in range(4):
                    nc.tensor.matmul(ps[:], wlk[:, kt * 256 + mt * 128: kt * 256 + (mt + 1) * 128],
                                     xpb[kt][:, ntk * 512:(ntk + 1) * 512], start=(kt == 0), stop=(kt == 3))
                nc.vector.tensor_scalar_add(lk[mt][:, ntk * 512:(ntk + 1) * 512], ps[:],
                                            bsf[:, 6 + mt:7 + mt])
        # lofi V^T aug
        for mc in range(8):
            ps = psA.tile([128, 512], F32, tag="ps", name="ps")
            for kt in range(4):
                nc.tensor.matmul(ps[:, 0:260], xpb[kt][:, mc * 128:(mc + 1) * 128],
                                 wlv[:, kt * 260:(kt + 1) * 260], start=(kt == 0), stop=(kt == 3))
            nc.vector.tensor_tensor(val[:, mc * 260:(mc + 1) * 260], ps[:, 0:260],
                                    blv_bc[:], mybir.AluOpType.add)

        # ---------- merged main loop ----------
        for nt in range(8):
            q0 = nt * 512
            fillers = []
            if nt == 0:
                fillers += [(hv_group, 0, sc) for sc in range(4)]
                fillers += [(qk_group, 1, mt) for mt in range(4)]
                fillers += [(lq_group, 1, mt) for mt in range(2)]
                fillers += [(hv_group, 1, sc) for sc in range(4)]
            if 1 <= nt < 7:
                fillers += [(qk_group, nt + 1, mt) for mt in range(4)]
                fillers += [(hv_group, nt + 1, sc) for sc in range(4)]
                fillers += [(lq_group, nt + 1, mt) for mt in range(2)]
            if nt >= 1:
                fillers += [(lproj_group, nt - 1, mt) for mt in range(2)]
            if nt >= 2:
                fillers += [(hproj_group, nt - 2, mt) for mt in range(2)]
            fi = 0

            def fill():
                nonlocal fi
                if fi < len(fillers):
                    f = fillers[fi]; fi += 1
                    f[0](*f[1:])

            for p in range(2):
                # lofi block: pair p, queries q0..q0+512, attn@V lag-2
                oacc = lO_p.tile([65, 1024], F32, tag="oac", name="oac")
                eSs = []
                for c in range(8):
                    sg = lS_p.tile([128, 1024], F32, tag="sg", name="sg")
                    for i in range(2):
                        r = 64 * i
                        nc.tensor.matmul(sg[:, i * 512:(i + 1) * 512],
                                         lk[p][r:r + 64, c * 128:(c + 1) * 128],
                                         lq[p][r:r + 64, q0:q0 + 512],
                                         start=True, stop=True)
                    eS = le_p.tile([128, 1024], BF16, tag="eS", name="eS")
                    nc.scalar.activation(eS[:], sg[:], AF.Exp, scale=SCALE)
                    eSs.append(eS)
                    if c >= 2:
                        _lofi_av(nc, oacc, val, eSs[c - 2], p, c - 2)
                    if c > 0:
                        fill()
                _lofi_av(nc, oacc, val, eSs[6], p, 6)
                fill()
                _lofi_av(nc, oacc, val, eSs[7], p, 7)
                stg = ls_p.tile([65, 1024], F32, tag="stg", name="stg")
                nc.vector.tensor_copy(stg[:], oacc[:])
                den_norm(stg, norm_l[p], p, q0, scatter=False)
                if p == 0 and nt >= 1:
                    # previous tile's hifi attention between the two lofi
                    # blocks: its exps extend the ACT wall while this tile's
                    # p1 exps cover the hifi tails' PE/DVE chains
                    eL0 = hifi_block(nt - 1, 0)
                    eL1 = hifi_block(nt - 1, 1)
                    hifi_tail(nt - 1, 0, eL0)
                    hifi_tail(nt - 1, 1, eL1)
        # tail: last hifi tile + remaining projections
        eL0 = hifi_block(7, 0)
        eL1 = hifi_block(7, 1)
        hifi_tail(7, 0, eL0)
        hifi_tail(7, 1, eL1)
        lproj_group(7, 0)
        lproj_group(7, 1)
        hproj_group(6, 0)
        hproj_group(6, 1)
        hproj_group(7, 0)
        hproj_group(7, 1)

    opx.release()
    oph.release()
    opl.release()
    wp.release()


def _lofi_av(nc, oacc, val, eS, p, c):
    for i in range(2):
        h = 2 * p + i
        nc.tensor.matmul(oacc[:, i * 512:(i + 1) * 512],
                         val[:, c * 260 + h * 65: c * 260 + (h + 1) * 65],
                         eS[:, i * 512:(i + 1) * 512],
                         start=(c == 0), stop=(c == 7))


def _prep_weights(W_hqkv, b_hqkv, W_hproj, b_hproj, W_lq, b_lq, W_lkv, b_lkv,
                  W_lproj, b_lproj):
    f = np.float32
    bf = ml_dtypes.bfloat16
    wqk = np.ascontiguousarray(np.asarray(W_hqkv)[:512].T, dtype=bf)
    bqk = np.ascontiguousarray(np.asarray(b_hqkv)[:512].reshape(4, 128).T, dtype=f)
    whv = np.zeros((512, 260), bf)
    bhv = np.zeros((1, 260), bf)
    for h in range(4):
        whv[:, 65 * h:65 * h + 64] = np.asarray(W_hqkv)[512 + 64 * h:512 + 64 * (h + 1)].T
        bhv[0, 65 * h:65 * h + 64] = np.asarray(b_hqkv)[512 + 64 * h:512 + 64 * (h + 1)]
        bhv[0, 65 * h + 64] = 1.0
    wlq = np.ascontiguousarray(np.asarray(W_lq).T, dtype=bf)
    blq = np.ascontiguousarray(np.asarray(b_lq).reshape(2, 128).T, dtype=f)
    wlk = np.ascontiguousarray((0.25 * np.asarray(W_lkv)[:256]).T, dtype=bf)
    blk = np.ascontiguousarray(np.asarray(b_lkv)[:256].reshape(2, 128).T, dtype=f)
    wlv = np.zeros((512, 260), bf)
    blv = np.zeros((1, 260), bf)
    for h in range(4):
        wlv[:, 65 * h:65 * h + 64] = 0.25 * np.asarray(W_lkv)[256 + 64 * h:256 + 64 * (h + 1)].T
        blv[0, 65 * h:65 * h + 64] = np.asarray(b_lkv)[256 + 64 * h:256 + 64 * (h + 1)]
        blv[0, 65 * h + 64] = 1.0
    # proj weights: bf16, transposed (in, out), pair-packed: rows 0-127 are the
    # pair's input channels; cols [p*256 + mt*128 ...] select (pair, out tile)
    whp = np.ascontiguousarray(
        np.asarray(W_hproj).T.reshape(2, 128, 256).transpose(1, 0, 2).reshape(128, 512), dtype=bf)
    bhp = np.ascontiguousarray(np.asarray(b_hproj).reshape(2, 128).T, dtype=f)
    wlp = np.ascontiguousarray(
        np.asarray(W_lproj).T.reshape(2, 128, 256).transpose(1, 0, 2).reshape(128, 512), dtype=bf)
    blp = np.ascontiguousarray(np.asarray(b_lproj).reshape(2, 128).T, dtype=f)
    mskL = np.zeros((128, 128), bf)
    mskR = np.zeros((128, 512), bf)
    for half in (0, 64):
        for g in range(32):
            mskL[half + g, 4 * g:4 * g + 4] = 1.0
            for t in range(4):
                mskR[half + g, 128 * t + 4 * g:128 * t + 4 * g + 4] = CBIG
        mskL[half + 32, :] = 1.0
        mskR[half + 32, :] = -CBIG
    smallf = np.concatenate([bqk, blq, blk, bhp, blp], axis=1).astype(f)
    smallb = np.zeros((128, 1160), bf)
    smallb[:, 0:128] = mskL
    smallb[:, 128:640] = mskR
    smallb[0, 640:900] = bhv[0]
    smallb[0, 900:1160] = blv[0]
    return dict(wqk=wqk, whv=whv, wlq=wlq, wlk=wlk, wlv=wlv, whp=whp,
                wlp=wlp, smallf=smallf, smallb=smallb)


def kernel(x, W_hqkv, b_hqkv, W_hproj, b_hproj, W_lq, b_lq, W_lkv, b_lkv,
           W_lproj, b_lproj, _trace=False):
    if "nc" not in _CACHE:
        _CACHE["nc"] = _build_bass()
    nc = _CACHE["nc"]
    wmap = _prep_weights(W_hqkv, b_hqkv, W_hproj, b_hproj, W_lq, b_lq,
                         W_lkv, b_lkv, W_lproj, b_lproj)
    x = np.asarray(x)
    B = x.shape[0]
    in_maps = []
    for b in range(8):
        m = dict(wmap)
        m["x"] = np.ascontiguousarray(x[b % B].reshape(512, N), dtype=ml_dtypes.bfloat16)
        in_maps.append(m)
    res = run_bass_kernel_spmd(nc, in_maps, core_ids=list(range(8)), trace=_trace)
    _CACHE["last_res"] = res
    y = np.stack([res.results[b]["y"].reshape(512, 64, 64) for b in range(B)])
    return y


# revision 25
# speedup vs baseline: 1.2350x; 1.2350x over previous
"""Trainium2 Bass kernel for dual-branch (hifi windowed + lofi downsampled-KV)
attention. Data-parallel over batch: 8 batches -> 8 NeuronCores.

Final: fully software-pipelined around the ACT exp stream (~19M softmax
elements at 1 elem/lane/cycle is the per-core floor). ~302us/core measured
(baseline 556us).

  Phase 0: DMA all x; 2x2 avg-pool (split DVE/gpsimd); hifi qk + V^T + lofi q
           for tile 0 (PE warm-up, only needs x); lofi k and V^T.
  Phase 1 (per 512-pixel tile nt): two lofi attention blocks (head pairs
           packed into [128,1024] PSUM via concurrent row-group matmuls, ONE
           exp per key chunk, attn@V lagged one chunk behind exp), with next
           tile's hifi/lofi projections and the previous tile's lofi output
           projection sprinkled between chunks as PE filler.
  Phase 2 (per 512-pixel block, pair-inner): hifi windowed attention
           (pair-packed logits+mask, one exp per block, attn@V and the
           normalize chain lagged one block), with the hifi output projection
           lagged one block-pair; projection evacuations on ACT (idle here).

  Softmax denominators ride as a ones-column in the V^T weights -> row 64 of
  the attn@V PSUM; packed via DMA transpose -> 128-lane reciprocal -> DMA
  back -> gpsimd partition_broadcast -> DVE normalize (fused with the hifi
  (g,i)->(h,w) scatter), writing pair-packed [128,N] tiles so the output
  projections contract over the full 128 partitions.
"""
import sys

sys.path.insert(0, "/opt/trn_rl_repo")

import numpy as np
import ml_dtypes

import concourse.bass as bass
import concourse.bacc as bacc
import concourse.mybir as mybir
import concourse.tile as tile
from concourse.bass_utils import run_bass_kernel_spmd

F32 = mybir.dt.float32
BF16 = mybir.dt.bfloat16
AF = mybir.ActivationFunctionType
MUL = mybir.AluOpType.mult

SCALE = 64 ** -0.5   # 0.125
N = 4096
M = 1024
CBIG = 320.0         # mask magnitude pre-scale (C/SCALE with C=40)

_CACHE = {}


def _build_bass():
    nc = bacc.Bacc("TRN2", target_bir_lowering=False, debug=False, num_devices=8)

    d = {}
    d["x_d"] = nc.dram_tensor("x", (512, N), BF16, kind="ExternalInput").ap()
    for nm, shp, dt in [
        ("wqk", (512, 512), BF16), ("whv", (512, 260), BF16),
        ("wlq", (512, 256), BF16), ("wlk", (512, 256), BF16),
        ("wlv", (512, 260), BF16), ("whp", (128, 512), BF16),
        ("wlp", (128, 512), BF16),
        ("smallf", (128, 12), F32), ("smallb", (128, 1160), BF16),
    ]:
        d[nm + "_d"] = nc.dram_tensor(nm, shp, dt, kind="ExternalInput").ap()
    d["y_d"] = nc.dram_tensor("y", (512, N), F32, kind="ExternalOutput").ap()

    with tile.TileContext(nc) as tc:
        _emit(nc, tc, d)
    nc.finalize()
    return nc


def _emit(nc, tc, d):
    x_d = d["x_d"]; y_d = d["y_d"]

    # ---- persistent: weights ----
    wp = tc.alloc_tile_pool(name="wp", bufs=1)
    wqk = wp.tile([128, 512 * 4], BF16, tag="wqk", name="wqk")
    whv = wp.tile([128, 260 * 4], BF16, tag="whv", name="whv")
    wlq = wp.tile([128, 256 * 4], BF16, tag="wlq", name="wlq")
    wlk = wp.tile([128, 256 * 4], BF16, tag="wlk", name="wlk")
    wlv = wp.tile([128, 260 * 4], BF16, tag="wlv", name="wlv")
    whp = wp.tile([128, 512], BF16, tag="whp", name="whp")
    wlp = wp.tile([128, 512], BF16, tag="wlp", name="wlp")
    bsf = wp.tile([128, 12], F32, tag="bsf", name="bsf")
    bsb = wp.tile([128, 1160], BF16, tag="bsb", name="bsb")
    ones1 = wp.tile([1, 128], BF16, tag="ones1", name="ones1")
    bqk = bsf[:, 0:4]; blq = bsf[:, 4:6]; blk = bsf[:, 6:8]
    bhp = bsf[:, 8:10]; blp = bsf[:, 10:12]
    mskL = bsb[:, 0:128]; mskR = bsb[:, 128:640]
    bhv = bsb[0:1, 640:900]; blv = bsb[0:1, 900:1160]
    bhv_bc = wp.tile([128, 260], BF16, tag="bhv_bc", name="bhv_bc")
    blv_bc = wp.tile([128, 260], BF16, tag="blv_bc", name="blv_bc")


    # ---- persistent pools, ordered by release time (LIFO stack) ----
    opl = tc.alloc_tile_pool(name="oplofi", bufs=1)       # lives until after lofi proj
    lq = [opl.tile([128, N], BF16, tag=f"lq{p}", name=f"lq{p}") for p in range(2)]
    lk = [opl.tile([128, M], BF16, tag=f"lk{p}", name=f"lk{p}") for p in range(2)]
    val = opl.tile([128, 260 * 8], BF16, tag="val", name="val")
    norm_l = [opl.tile([128, N], BF16, tag=f"nl{p}", name=f"nl{p}") for p in range(2)]
    oph = tc.alloc_tile_pool(name="ophifi", bufs=1)       # lives until after hifi proj
    qkh = [oph.tile([128, N], BF16, tag=f"qkh{p}", name=f"qkh{p}") for p in range(4)]
    vah = oph.tile([128, 260 * 32], BF16, tag="vah", name="vah")
    norm_h = [oph.tile([128, N], BF16, tag=f"nh{p}", name=f"nh{p}") for p in range(2)]
    opx = tc.alloc_tile_pool(name="opx", bufs=1)          # x tiles; through phase 1
    xbig = [opx.tile([128, N], BF16, tag=f"xb{kt}", name=f"xb{kt}") for kt in range(4)]
    xpb = [opx.tile([128, M], BF16, tag=f"xpb{kt}", name=f"xpb{kt}") for kt in range(4)]
    # DMA order matters: the Sync engine serializes dma_start issues at
    # ~0.7us each and the DMA engines drain ~in issue order.  Small bias/mask
    # tensors first (ACT's first evacuations need bqk), then the warm-up
    # weights, then the big x transfers, then the weights not needed until
    # later.
    nc.sync.dma_start(bsf[:], d["smallf_d"][:, :])
    nc.sync.dma_start(bsb[:], d["smallb_d"][:, :])
    for (t, nm) in [(wqk, "wqk"), (whv, "whv"), (wlq, "wlq")]:
        dr = d[nm + "_d"]
        w = dr.shape[1]
        nc.sync.dma_start(t[:].rearrange("p (k w) -> p k w", k=4),
                          dr.rearrange("(k p) w -> p k w", k=4))
    for kt in range(4):
        nc.sync.dma_start(xbig[kt][:], x_d[kt * 128:(kt + 1) * 128, :])
    xb = [[xbig[kt][:, nt * 512:(nt + 1) * 512] for kt in range(4)] for nt in range(8)]
    for (t, nm) in [(wlk, "wlk"), (wlv, "wlv")]:
        dr = d[nm + "_d"]
        w = dr.shape[1]
        nc.sync.dma_start(t[:].rearrange("p (k w) -> p k w", k=4),
                          dr.rearrange("(k p) w -> p k w", k=4))
    for (t, nm) in [(whp, "whp"), (wlp, "wlp")]:
        nc.sync.dma_start(t[:], d[nm + "_d"][:, :])
    nc.vector.memset(ones1[:], 1.0)
    nc.gpsimd.partition_broadcast(bhv_bc[:], bsb[0:1, 640:900])
    nc.gpsimd.partition_broadcast(blv_bc[:], bsb[0:1, 900:1160])

    # ============ single merged pipeline: one PSUM footprint ============
    # psA (2 banks) projection groups | sg-tag (4 banks) lofi S / hifi logits
    # | oac-tag (2 banks) lofi attn@V acc / hifi attn@V.  ACT runs the exp
    # wall; everything else is filler around it.
    with tc.tile_pool(name="psA", bufs=2, space="PSUM") as psA, \
         tc.tile_pool(name="lS", bufs=2, space="PSUM") as lS_p, \
         tc.tile_pool(name="lO", bufs=1, space="PSUM") as lO_p, \
         tc.tile_pool(name="p0t1", bufs=2) as t1_p, \
         tc.tile_pool(name="lexp", bufs=4) as le_p, \
         tc.tile_pool(name="lstg", bufs=2) as ls_p, \
         tc.tile_pool(name="lpk", bufs=2) as lpk_p, \
         tc.tile_pool(name="lrb", bufs=2) as lrb_p, \
         tc.tile_pool(name="lpyb", bufs=2) as lyb_p:

        def qk_group(nt, mt, warm=False):
            ps = psA.tile([128, 512], F32, tag="ps", name="ps")
            for kt in range(4):
                nc.tensor.matmul(ps[:], wqk[:, kt * 512 + mt * 128: kt * 512 + (mt + 1) * 128],
                                 xb[nt][kt], start=(kt == 0), stop=(kt == 3))
            if warm:
                nc.scalar.activation(qkh[mt][:, nt * 512:(nt + 1) * 512], ps[:],
                                     AF.Identity, bias=bsf[:, mt:mt + 1], scale=1.0)
            else:
                nc.vector.tensor_scalar_add(qkh[mt][:, nt * 512:(nt + 1) * 512], ps[:],
                                            bsf[:, mt:mt + 1])

        def hv_group(nt, sc, warm=False):
            st = nt * 4 + sc
            ps = psA.tile([128, 512], F32, tag="ps", name="ps")
            for kt in range(4):
                nc.tensor.matmul(ps[:, 0:260], xbig[kt][:, nt * 512 + sc * 128: nt * 512 + (sc + 1) * 128],
                                 whv[:, kt * 260:(kt + 1) * 260], start=(kt == 0),
                                 stop=(kt == 3 and not warm))
            if warm:
                nc.tensor.matmul(ps[:, 0:260], ones1[:], bsb[0:1, 640:900],
                                 start=False, stop=True, skip_group_check=True)
                nc.scalar.activation(vah[:, st * 260:(st + 1) * 260], ps[:, 0:260],
                                     AF.Identity, scale=1.0)
            else:
                nc.vector.tensor_tensor(vah[:, st * 260:(st + 1) * 260], ps[:, 0:260],
                                        bhv_bc[:], mybir.AluOpType.add)

        def lq_group(nt, mt, warm=False):
            ps = psA.tile([128, 512], F32, tag="ps", name="ps")
            for kt in range(4):
                nc.tensor.matmul(ps[:], wlq[:, kt * 256 + mt * 128: kt * 256 + (mt + 1) * 128],
                                 xb[nt][kt], start=(kt == 0), stop=(kt == 3))
            if warm:
                nc.scalar.activation(lq[mt][:, nt * 512:(nt + 1) * 512], ps[:],
                                     AF.Identity, bias=bsf[:, 4 + mt:5 + mt], scale=1.0)
            else:
                nc.vector.tensor_scalar_add(lq[mt][:, nt * 512:(nt + 1) * 512], ps[:],
                                            bsf[:, 4 + mt:5 + mt])

        def lproj_group(nt, mt):
            ps = psA.tile([128, 512], F32, tag="ps", name="ps")
            for p in range(2):
                nc.tensor.matmul(ps[:],
                                 wlp[:, p * 256 + mt * 128: p * 256 + (mt + 1) * 128],
                                 norm_l[p][:, nt * 512:(nt + 1) * 512],
                                 start=(p == 0), stop=(p == 1))
            yb = lyb_p.tile([128, 512], F32, tag="yb", name="yb")
            nc.vector.tensor_scalar_add(yb[:], ps[:], bsf[:, 10 + mt:11 + mt])
            nc.sync.dma_start(y_d[256 + mt * 128: 256 + (mt + 1) * 128,
                                  nt * 512:(nt + 1) * 512], yb[:])

        def hproj_group(nt, mt):
            ps = psA.tile([128, 512], F32, tag="ps", name="ps")
            for p in range(2):
                nc.tensor.matmul(ps[:],
                                 whp[:, p * 256 + mt * 128: p * 256 + (mt + 1) * 128],
                                 norm_h[p][:, nt * 512:(nt + 1) * 512],
                                 start=(p == 0), stop=(p == 1))
            yb = lyb_p.tile([128, 512], F32, tag="yb", name="yb")
            nc.vector.tensor_scalar_add(yb[:], ps[:], bsf[:, 8 + mt:9 + mt])
            nc.sync.dma_start(y_d[mt * 128:(mt + 1) * 128, nt * 512:(nt + 1) * 512], yb[:])

        def den_norm(stg, dst_norm, p, col0, scatter):
            # pack denominators -> 128-lane reciprocal -> broadcast -> multiply
            dpk = lpk_p.tile([128, 8], F32, tag="dpk", name="dpk")
            nc.sync.dma_start(dpk[:], stg[64:65, :].rearrange("o (p f) -> o p f", f=8))
            rpk = lpk_p.tile([128, 8], F32, tag="rpk", name="rpk")
            nc.vector.reciprocal_approx_fast(rpk[:], dpk[:])
            rrow = lpk_p.tile([1, 1024], F32, tag="rrow", name="rrow")
            nc.sync.dma_start(rrow[:].rearrange("o (p f) -> o p f", f=8), rpk[:])
            rb = lrb_p.tile([64, 1024], F32, tag="rb", name="rb")
            nc.gpsimd.partition_broadcast(rb[:], rrow[:])
            for i in range(2):
                if scatter:
                    dstv = dst_norm[64 * i:64 * i + 64, col0:col0 + 512].rearrange(
                        "p (s i1 gi2) -> p s i1 gi2", s=4, i1=2, gi2=64)
                    srcv = stg[0:64, i * 512:(i + 1) * 512].rearrange(
                        "p (s g i1 i2) -> p s g i1 i2", s=4, g=32, i1=2, i2=2)
                    rbv = rb[:, i * 512:(i + 1) * 512].rearrange(
                        "p (s g i1 i2) -> p s g i1 i2", s=4, g=32, i1=2, i2=2)
                    for i1 in range(2):
                        nc.vector.tensor_tensor(dstv[:, :, i1, :], srcv[:, :, :, i1, :],
                                                rbv[:, :, :, i1, :], MUL)
                else:
                    nc.vector.tensor_tensor(dst_norm[64 * i:64 * i + 64, col0:col0 + 512],
                                            stg[0:64, i * 512:(i + 1) * 512],
                                            rb[:, i * 512:(i + 1) * 512], MUL)

        def hifi_block(sgh, p):
            qt_, kt_ = qkh[p], qkh[2 + p]
            Lg = lS_p.tile([128, 1024], F32, tag="sg", name="Lg")
            for s4 in range(4):
                st = sgh * 4 + s4
                for i in range(2):
                    r = 64 * i
                    co = i * 512 + s4 * 128
                    nc.tensor.matmul(Lg[:, co:co + 128],
                                     kt_[r:r + 64, st * 128:(st + 1) * 128],
                                     qt_[r:r + 64, st * 128:(st + 1) * 128],
                                     start=True, stop=False)
                    nc.tensor.matmul(Lg[:, co:co + 128],
                                     bsb[r:r + 64, 0:128], bsb[r:r + 64, 128:256],
                                     start=False, stop=True)
            eL = le_p.tile([128, 1024], BF16, tag="eS", name="eL")
            nc.scalar.activation(eL[:], Lg[:], AF.Exp, scale=SCALE)
            return eL

        def hifi_tail(sgh, p, eL):
            oh = lO_p.tile([65, 1024], F32, tag="oac", name="oh")
            for s4 in range(4):
                st = sgh * 4 + s4
                for i in range(2):
                    h = 2 * p + i
                    co = i * 512 + s4 * 128
                    nc.tensor.matmul(oh[:, co:co + 128],
                                     vah[:, st * 260 + h * 65: st * 260 + (h + 1) * 65],
                                     eL[:, co:co + 128], start=True, stop=True)
            stg = ls_p.tile([65, 1024], F32, tag="stg", name="stg")
            nc.vector.tensor_copy(stg[:], oh[:])
            den_norm(stg, norm_h[p], p, sgh * 512, scatter=True)

        # ---------- Phase 0 ----------
        # 2x2 avg-pool in bf16 (the /4 folded into wlk/wlv); split DVE/gpsimd
        for nt in range(8):
            for kt in range(4):
                eng = nc.vector if (nt * 4 + kt) % 8 < 5 else nc.gpsimd
                v = xb[nt][kt][:].rearrange("p (h w2 two) -> p h w2 two", w2=32, two=2)
                t1 = t1_p.tile([128, 256], BF16, tag=f"t1{kt % 2}", name="t1")
                t1v = t1[:].rearrange("p (h w2) -> p h w2", w2=32)
                eng.tensor_add(t1v, v[:, :, :, 0], v[:, :, :, 1])
                t1p = t1[:].rearrange("p (i two w2) -> p i two w2", two=2, w2=32)
                xpv = xpb[kt][:, nt * 128:(nt + 1) * 128].rearrange("p (i w2) -> p i w2", w2=32)
                eng.tensor_add(xpv, t1p[:, :, 0, :], t1p[:, :, 1, :])
        # PE warm-up: tiles 0-1 projections (evacuate via ACT, idle pre-wall;
        # DVE is busy pooling)
        for wnt in range(2):
            for mt in range(4):
                qk_group(wnt, mt, warm=True)
            for sc in range(4):
                hv_group(wnt, sc, warm=True)
            for mt in range(2):
                lq_group(wnt, mt, warm=True)
        # lofi k
        for mt in range(2):
            for ntk in range(2):
                ps = psA.tile([128, 512], F32, tag="ps", name="ps")
                for kt in range(4):
                    nc.tensor.matmul(ps[:], wlk[:, kt * 256 + mt * 128: kt * 256 + (mt + 1) * 128],
                                     xpb[kt][:, ntk * 512:(ntk + 1) * 512], start=(kt == 0), stop=(kt == 3))
                nc.vector.tensor_scalar_add(lk[mt][:, ntk * 512:(ntk + 1) * 512], ps[:],
                                            bsf[:, 6 + mt:7 + mt])
        # lofi V^T aug
        for mc in range(8):
            ps = psA.tile([128, 512], F32, tag="ps", name="ps")
            for kt in range(4):
                nc.tensor.matmul(ps[:, 0:260], xpb[kt][:, mc * 128:(mc + 1) * 128],
                                 wlv[:, kt * 260:(kt + 1) * 260], start=(kt == 0), stop=(kt == 3))
            nc.vector.tensor_tensor(val[:, mc * 260:(mc + 1) * 260], ps[:, 0:260],
                                    blv_bc[:], mybir.AluOpType.add)

        # ---------- merged main loop ----------
        for nt in range(8):
            q0 = nt * 512
            fillers = []
            if nt < 6:
                fillers += [(qk_group, nt + 2, mt) for mt in range(4)]
                fillers += [(hv_group, nt + 2, sc) for sc in range(4)]
                fillers += [(lq_group, nt + 2, mt) for mt in range(2)]
            if nt >= 1:
                fillers += [(lproj_group, nt - 1, mt) for mt in range(2)]
            if nt >= 2:
                fillers += [(hproj_group, nt - 1, mt) for mt in range(2)]
            fi = 0

            def fill():
                nonlocal fi
                if fi < len(fillers):
                    f = fillers[fi]; fi += 1
                    f[0](*f[1:])

            for p in range(2):
                # lofi block: pair p, queries q0..q0+512, attn@V lag-2
                oacc = lO_p.tile([65, 1024], F32, tag="oac", name="oac")
                eSs = []
                for c in range(8):
                    sg = lS_p.tile([128, 1024], F32, tag="sg", name="sg")
                    for i in range(2):
                        r = 64 * i
                        nc.tensor.matmul(sg[:, i * 512:(i + 1) * 512],
                                         lk[p][r:r + 64, c * 128:(c + 1) * 128],
                                         lq[p][r:r + 64, q0:q0 + 512],
                                         start=True, stop=True)
                    eS = le_p.tile([128, 1024], BF16, tag="eS", name="eS")
                    nc.scalar.activation(eS[:], sg[:], AF.Exp, scale=SCALE)
                    eSs.append(eS)
                    if c >= 2:
                        _lofi_av(nc, oacc, val, eSs[c - 2], p, c - 2)
                    if c > 0:
                        fill()
                _lofi_av(nc, oacc, val, eSs[6], p, 6)
                fill()
                _lofi_av(nc, oacc, val, eSs[7], p, 7)
                stg = ls_p.tile([65, 1024], F32, tag="stg", name="stg")
                nc.vector.tensor_copy(stg[:], oacc[:])
                den_norm(stg, norm_l[p], p, q0, scatter=False)
            eL0 = hifi_block(nt, 0)
            eL1 = hifi_block(nt, 1)
            hifi_tail(nt, 0, eL0)
            hifi_tail(nt, 1, eL1)

        # tail: remaining projections only (hifi tile 7 already ran at nt=7)
        lproj_group(7, 0)
        lproj_group(7, 1)
        hproj_group(0, 0)
        hproj_group(0, 1)
        hproj_group(7, 0)
        hproj_group(7, 1)

    opx.release()
    oph.release()
    opl.release()
    wp.release()


def _lofi_av(nc, oacc, val, eS, p, c):
    for i in range(2):
        h = 2 * p + i
        nc.tensor.matmul(oacc[:, i * 512:(i + 1) * 512],
                         val[:, c * 260 + h * 65: c * 260 + (h + 1) * 65],
                         eS[:, i * 512:(i + 1) * 512],
                         start=(c == 0), stop=(c == 7))


def _prep_weights(W_hqkv, b_hqkv, W_hproj, b_hproj, W_lq, b_lq, W_lkv, b_lkv,
                  W_lproj, b_lproj):
    f = np.float32
    bf = ml_dtypes.bfloat16
    wqk = np.ascontiguousarray(np.asarray(W_hqkv)[:512].T, dtype=bf)
    bqk = np.ascontiguousarray(np.asarray(b_hqkv)[:512].reshape(4, 128).T, dtype=f)
    whv = np.zeros((512, 260), bf)
    bhv = np.zeros((1, 260), bf)
    for h in range(4):
        whv[:, 65 * h:65 * h + 64] = np.asarray(W_hqkv)[512 + 64 * h:512 + 64 * (h + 1)].T
        bhv[0, 65 * h:65 * h + 64] = np.asarray(b_hqkv)[512 + 64 * h:512 + 64 * (h + 1)]
        bhv[0, 65 * h + 64] = 1.0
    wlq = np.ascontiguousarray(np.asarray(W_lq).T, dtype=bf)
    blq = np.ascontiguousarray(np.asarray(b_lq).reshape(2, 128).T, dtype=f)
    wlk = np.ascontiguousarray((0.25 * np.asarray(W_lkv)[:256]).T, dtype=bf)
    blk = np.ascontiguousarray(np.asarray(b_lkv)[:256].reshape(2, 128).T, dtype=f)
    wlv = np.zeros((512, 260), bf)
    blv = np.zeros((1, 260), bf)
    for h in range(4):
        wlv[:, 65 * h:65 * h + 64] = 0.25 * np.asarray(W_lkv)[256 + 64 * h:256 + 64 * (h + 1)].T
        blv[0, 65 * h:65 * h + 64] = np.asarray(b_lkv)[256 + 64 * h:256 + 64 * (h + 1)]
        blv[0, 65 * h + 64] = 1.0
    # proj weights: bf16, transposed (in, out), pair-packed: rows 0-127 are the
    # pair's input channels; cols [p*256 + mt*128 ...] select (pair, out tile)
    whp = np.ascontiguousarray(
        np.asarray(W_hproj).T.reshape(2, 128, 256).transpose(1, 0, 2).reshape(128, 512), dtype=bf)
    bhp = np.ascontiguousarray(np.asarray(b_hproj).reshape(2, 128).T, dtype=f)
    wlp = np.ascontiguousarray(
        np.asarray(W_lproj).T.reshape(2, 128, 256).transpose(1, 0, 2).reshape(128, 512), dtype=bf)
    blp = np.ascontiguousarray(np.asarray(b_lproj).reshape(2, 128).T, dtype=f)
    mskL = np.zeros((128, 128), bf)
    mskR = np.zeros((128, 512), bf)
    for half in (0, 64):
        for g in range(32):
            mskL[half + g, 4 * g:4 * g + 4] = 1.0
            for t in range(4):
                mskR[half + g, 128 * t + 4 * g:128 * t + 4 * g + 4] = CBIG
        mskL[half + 32, :] = 1.0
        mskR[half + 32, :] = -CBIG
    smallf = np.concatenate([bqk, blq, blk, bhp, blp], axis=1).astype(f)
    smallb = np.zeros((128, 1160), bf)
    smallb[:, 0:128] = mskL
    smallb[:, 128:640] = mskR
    smallb[0, 640:900] = bhv[0]
    smallb[0, 900:1160] = blv[0]
    return dict(wqk=wqk, whv=whv, wlq=wlq, wlk=wlk, wlv=wlv, whp=whp,
                wlp=wlp, smallf=smallf, smallb=smallb)


def kernel(x, W_hqkv, b_hqkv, W_hproj, b_hproj, W_lq, b_lq, W_lkv, b_lkv,
           W_lproj, b_lproj, _trace=False):
    if "nc" not in _CACHE:
        _CACHE["nc"] = _build_bass()
    nc = _CACHE["nc"]
    wmap = _prep_weights(W_hqkv, b_hqkv, W_hproj, b_hproj, W_lq, b_lq,
                         W_lkv, b_lkv, W_lproj, b_lproj)
    x = np.asarray(x)
    B = x.shape[0]
    in_maps = []
    for b in range(8):
        m = dict(wmap)
        m["x"] = np.ascontiguousarray(x[b % B].reshape(512, N), dtype=ml_dtypes.bfloat16)
        in_maps.append(m)
    res = run_bass_kernel_spmd(nc, in_maps, core_ids=list(range(8)), trace=_trace)
    _CACHE["last_res"] = res
    y = np.stack([res.results[b]["y"].reshape(512, 64, 64) for b in range(B)])
    return y
